# revision 93
# baseline (speedup 1.0000x reference)
import sys
sys.path.insert(0, "/opt/trn_rl_repo")
import numpy as np
import ml_dtypes

V, T, D, H, L = 50257, 512, 512, 8, 6
B = 4
HD = 64
VH = 25129          # ceil(V/2); half 1 is zero-padded to VH
NC_CHUNKS = 50      # 50*512 = 25600 >= VH
EPS = 1e-5
NEG = -1.0e9

nbf = ml_dtypes.bfloat16
_NC = {}
LAST = None


def _build_nc(has_bias=True):
    import concourse.bass as bass
    import concourse.tile as tile
    from concourse import mybir
    from concourse.masks import make_identity

    F32 = mybir.dt.float32
    BF16 = mybir.dt.bfloat16
    nc = bass.Bass()

    h0_in = nc.declare_dram_parameter("h0", [T, D], F32, isOutput=False)
    u0T_in = nc.declare_dram_parameter("u0T", [128, 4 * T], BF16, isOutput=False)
    wqk_in = nc.declare_dram_parameter("wqk", [L, 128, 4096], BF16, isOutput=False)
    wv_in = nc.declare_dram_parameter("wv", [L, 128, 2048], BF16, isOutput=False)
    wo_in = nc.declare_dram_parameter("wo", [L, 128, 2048], BF16, isOutput=False)
    wf1_in = nc.declare_dram_parameter("wf1", [L, 128, 8192], BF16, isOutput=False)
    wf2_in = nc.declare_dram_parameter("wf2", [L, 128, 8192], BF16, isOutput=False)
    qkb_in = nc.declare_dram_parameter("qkb", [128, 8 * L], F32, isOutput=False)
    f1b_in = nc.declare_dram_parameter("f1b", [128, 16 * L], F32, isOutput=False)
    rowb_in = (nc.declare_dram_parameter("rowb", [1, 1536 * L], BF16, isOutput=False)
               if has_bias else None)
    maskt_in = nc.declare_dram_parameter("maskt", [128, 128], BF16, isOutput=False)
    hw_in = nc.declare_dram_parameter("hw", [NC_CHUNKS, 128, 2048], BF16, isOutput=False)
    out = nc.declare_dram_parameter("logits", [T, NC_CHUNKS * 512], BF16, isOutput=True)

    EXP = mybir.ActivationFunctionType.Exp
    GELU = mybir.ActivationFunctionType.Gelu
    IDN = mybir.ActivationFunctionType.Identity
    SQRT = mybir.ActivationFunctionType.Sqrt

    with tile.TileContext(nc) as tc:
        with (
            tc.tile_pool(name="pers", bufs=1) as pers,
            tc.tile_pool(name="wpool", bufs=2) as wpool,
            tc.tile_pool(name="wpool1", bufs=1) as wpool1,
            tc.tile_pool(name="act", bufs=1) as act,
            tc.tile_pool(name="sm", bufs=3) as sm,
            tc.tile_pool(name="exp_pool", bufs=4) as exp_pool,
            tc.tile_pool(name="hwp", bufs=4) as hwp,
            tc.tile_pool(name="stg", bufs=8) as stg,
            tc.tile_pool(name="drp", bufs=4, space="DRAM") as drp,
            tc.tile_pool(name="ps_mm", bufs=2, space="PSUM") as ps_mm,
            tc.tile_pool(name="ps_st", bufs=2, space="PSUM") as ps_st,
            tc.tile_pool(name="ps_ot", bufs=2, space="PSUM") as ps_ot,
            tc.tile_pool(name="ps_tr", bufs=1, space="PSUM") as ps_tr,
            tc.tile_pool(name="ps_bc", bufs=1, space="PSUM") as ps_bc,
        ):
            # ---- persistent constants ----
            ident = pers.tile([128, 128], BF16, tag="ident")
            make_identity(nc, ident)
            ones1 = pers.tile([1, 128], BF16, tag="ones1")
            nc.vector.memset(ones1, 1.0)
            epst = pers.tile([128, 1], F32, tag="epst")
            nc.vector.memset(epst, EPS)
            u0big = act.tile([128, 4 * T], BF16, tag="u0big")
            nc.sync.dma_start(out=u0big, in_=u0T_in[:])
            maskt = pers.tile([128, 128], BF16, tag="maskt")
            nc.sync.dma_start(out=maskt, in_=maskt_in[:])
            qkb = pers.tile([128, 8 * L], F32, tag="qkb")
            nc.sync.dma_start(out=qkb, in_=qkb_in[:])
            f1b = pers.tile([128, 16 * L], F32, tag="f1b")
            nc.sync.dma_start(out=f1b, in_=f1b_in[:])
            if has_bias:
                rowb = pers.tile([1, 1536 * L], BF16, tag="rowb")
                nc.sync.dma_start(out=rowb, in_=rowb_in[:])
            junk_sb = pers.tile([1, 8], F32, tag="junk_sb")

            # residual + v_ext persistent (h0 DMAs deferred into layer 0 --
            # first h use is the layer-0 out-proj residual)
            h = [pers.tile([128, D], F32, tag=f"h{i}", name=f"h{i}") for i in range(4)]
            vext = [pers.tile([128, 520], BF16, tag=f"vext{i}", name=f"vext{i}") for i in range(4)]
            for i in range(4):
                nc.vector.memset(
                    vext[i].rearrange("p (h c) -> p h c", h=H)[:, :, 64:65], 1.0)

            # pre-touches: ACT reads the bias tiles once; PE touches dma'd mats
            nc.scalar.copy(out=junk_sb[0:1, 0:1], in_=qkb[0:1, 0:1])
            nc.scalar.copy(out=junk_sb[0:1, 1:2], in_=f1b[0:1, 1:2])

            def pe_touch(ap):
                jk = ps_mm.tile([1, 8], F32, tag="pmm")
                nc.tensor.matmul(jk[0:1, 0:1], ap[:, 0:1], ap[:, 0:1],
                                 start=True, stop=True, skip_group_check=True)

            if has_bias:
                pe_touch(rowb)

            PTAG = {id(ps_mm): "pmm", id(ps_st): "st", id(ps_ot): "po", id(ps_tr): "tp"}

            def ps_rot(pools, n):
                # rotate full-bank [128,D] f32 psum tiles across idle pools
                pool = pools[n % len(pools)]
                return pool.tile([128, D], F32, tag=PTAG[id(pool)], name="prot")

            LN_ = mybir.ActivationFunctionType.Ln
            # NOTE: GPSIMD/Pool cannot access PSUM on HW — PSUM-sourced
            # copies must go to DVE or ACT.
            cp_engines = [
                lambda o, i_: nc.vector.tensor_copy(o, i_),
                lambda o, i_: nc.scalar.copy(out=o, in_=i_),
            ]

            def ln_chunk(i, utag, uT, pp=None):
                # h[i] (+ optional residual pp) -> normalized, transposed
                # into uT. rstd = exp(-0.5*ln(var+eps)) keeps ACT on one
                # function table; the normalize is split DVE/ACT so the
                # first transposes start earlier.
                if pp is not None:
                    nc.vector.tensor_add(h[i], h[i], pp)
                src = h[i]
                stats = sm.tile([128, 6], F32, tag="stats")
                mv = sm.tile([128, 2], F32, tag="mv")
                nc.vector.bn_stats(out=stats, in_=src)
                nc.vector.bn_aggr(out=mv, in_=stats)
                vin = mv[:, 1:2]
                lnv = sm.tile([128, 1], F32, tag="lnv")
                nc.scalar.activation(out=lnv, in_=vin, func=LN_,
                                     bias=epst, scale=1.0)
                rstd = sm.tile([128, 1], F32, tag="rstd")
                nc.scalar.activation(out=rstd, in_=lnv, func=EXP, scale=-0.5)
                u = act.tile([128, D], BF16, tag=f"{utag}{i}")
                nc.vector.tensor_scalar(
                    out=u[:, 0:256], in0=src[:, 0:256], scalar1=mv[:, 0:1],
                    scalar2=rstd,
                    op0=mybir.AluOpType.subtract, op1=mybir.AluOpType.mult)
                nmr = sm.tile([128, 1], F32, tag="nmr")
                nc.vector.tensor_scalar(
                    out=nmr, in0=mv[:, 0:1], scalar1=rstd, scalar2=-1.0,
                    op0=mybir.AluOpType.mult, op1=mybir.AluOpType.mult)
                nc.scalar.activation(out=u[:, 256:512], in_=src[:, 256:512],
                                     func=IDN, bias=nmr, scale=rstd)
                for k in range(4):
                    # alternate PSUM banks (ps_bc is idle outside attention)
                    # so transpose k+1 does not wait on copy k
                    pool, tag = (ps_tr, "tp") if k % 2 == 0 else (ps_bc, "pbc")
                    tp = pool.tile([128, 128], BF16, tag=tag)
                    nc.tensor.transpose(tp, u[:, k * 128:(k + 1) * 128], ident)
                    cp_engines[k % 2](uT[k][:, i * 128:(i + 1) * 128], tp)

            # layer-0 uT comes pre-normalized+transposed from the host
            uT = [u0big[:, k * T:(k + 1) * T] for k in range(4)]

            for l in range(L):
                # ---- stream layer weights (SP queue; prefetch-friendly) ----
                wv = wpool1.tile([128, 2048], BF16, tag="wv")
                nc.sync.dma_start(out=wv, in_=wv_in[l])
                wqk = wpool.tile([128, 4096], BF16, tag="wqk")
                nc.sync.dma_start(out=wqk, in_=wqk_in[l])
                wo = wpool1.tile([128, 2048], BF16, tag="wo")
                nc.sync.dma_start(out=wo, in_=wo_in[l])
                wf1 = wpool1.tile([128, 8192], BF16, tag="wf1")
                nc.sync.dma_start(out=wf1, in_=wf1_in[l])
                wf2 = wpool1.tile([128, 8192], BF16, tag="wf2")
                nc.sync.dma_start(out=wf2, in_=wf2_in[l])
                if l == 0:
                    for i in range(4):
                        nc.sync.dma_start(out=h[i], in_=h0_in[i * 128:(i + 1) * 128, :])

                # ---- v (natural) -> vext ----
                # layer 0: the initial LN chunks interleave with the v chains
                # (v chain i needs only chunk i's transposed columns)
                pe_touch(wv)
                for i in range(4):
                    pv = ps_mm.tile([128, T], F32, tag="pmm")
                    for k in range(4):
                        nc.tensor.matmul(pv, uT[k][:, i * 128:(i + 1) * 128],
                                         wv[:, k * 512:(k + 1) * 512],
                                         start=(k == 0),
                                         stop=(not has_bias and k == 3))
                    if has_bias:
                        nc.tensor.matmul(pv, ones1, rowb[:, 1536 * l:1536 * l + 512],
                                         start=False, stop=True)
                    nc.vector.tensor_copy(
                        vext[i].rearrange("p (h c) -> p h c", h=H)[:, :, 0:64],
                        pv[:].rearrange("p (h c) -> p h c", h=H))

                # ---- qk chains interleaved with attention heads ----
                OTs = [act.tile([128, T], BF16, tag=f"ots{k}", name=f"ots{k}l") for k in range(4)]
                qTs, kTs = [None] * 4, [None] * 4

                qkn = [0]

                def qk_chain(m):
                    pq = ps_rot([ps_mm, ps_tr], qkn[0]); qkn[0] += 1
                    for k in range(4):
                        nc.tensor.matmul(pq, wqk[:, (k * 8 + m) * 128:(k * 8 + m + 1) * 128],
                                         uT[k], start=(k == 0), stop=(k == 3))
                    dst = act.tile([128, T], BF16, tag=f"qk{m}")
                    if m % 4 < 2:
                        # ACT is idle early in attention; DVE is chain-bound
                        nc.scalar.activation(out=dst, in_=pq, func=IDN,
                                             bias=qkb[:, 8 * l + m:8 * l + m + 1],
                                             scale=1.0)
                    else:
                        nc.vector.tensor_scalar_add(out=dst, in0=pq,
                                                    scalar1=qkb[:, 8 * l + m:8 * l + m + 1])
                    if m < 4:
                        qTs[m] = dst
                    else:
                        kTs[m - 4] = dst

                def attn_scores(hh):
                    # score blocks j=1 (384 cols) and j=3 (128 cols) share one
                    # PSUM tile / one exp call: 3 exps per head instead of 4.
                    m, base = hh // 2, (hh % 2) * 64
                    kT, qT = kTs[m], qTs[m]

                    def sc(st, col0, j):
                        nj = T - j * 128
                        nc.tensor.matmul(st[:, col0:col0 + nj],
                                         kT[base:base + 64, j * 128:(j + 1) * 128],
                                         qT[base:base + 64, j * 128:T],
                                         start=True, stop=False)
                        nc.tensor.matmul(st[:, col0:col0 + 128], ident, maskt,
                                         start=False, stop=True)

                    st0 = ps_st.tile([128, T], F32, tag="st")
                    sc(st0, 0, 0)
                    st1 = ps_st.tile([128, T], F32, tag="st")
                    sc(st1, 0, 1)
                    st23 = ps_st.tile([128, T], F32, tag="st")
                    sc(st23, 0, 2)
                    sc(st23, 256, 3)
                    ex0 = exp_pool.tile([128, T], BF16, tag="ex0", name="ex0")
                    nc.scalar.activation(out=ex0, in_=st0, func=EXP, scale=0.125)
                    ex1 = exp_pool.tile([128, 384], BF16, tag="ex1", name="ex1")
                    nc.scalar.activation(out=ex1, in_=st1[:, 0:384], func=EXP, scale=0.125)
                    ex23 = exp_pool.tile([128, 384], BF16, tag="ex23", name="ex23")
                    nc.scalar.activation(out=ex23, in_=st23[:, 0:384], func=EXP, scale=0.125)
                    return ex0, ex1, ex23

                def attn_av(hh, ex):
                    # av matmuls; stage O to SBUF early (frees the po bank),
                    # broadcast 1/denom via a DRAM round-trip DMA (off-engine)
                    ex0, ex1, ex23 = ex
                    rhs = [ex0, ex1, ex23[:, 0:256], ex23[:, 256:384]]
                    po = ps_ot.tile([65, T], F32, tag="po")
                    for j in range(4):
                        nc.tensor.matmul(po[:, j * 128:T],
                                         vext[j][:, 65 * hh:65 * hh + 65],
                                         rhs[j],
                                         start=(j == 0), stop=(j == 3))
                    rs = sm.tile([1, T], BF16, tag="rs")
                    with nc.allow_low_precision("softmax denom recip in bf16"):
                        nc.vector.reciprocal(out=rs, in_=po[64:65, :])
                    posb = exp_pool.tile([64, T], BF16, tag="posb")
                    nc.vector.tensor_copy(posb, po[0:64, :])
                    dsc = drp.tile([1, T], BF16, tag="dsc")
                    nc.sync.dma_start(out=dsc, in_=rs)
                    rb = exp_pool.tile([64, T], BF16, tag="rb")
                    nc.sync.dma_start(out=rb, in_=dsc.to_broadcast([64, T]))
                    return posb, rb

                def attn_div(hh, posb, rb):
                    # all-bf16 SBUF multiply (4x DVE mode)
                    m, base = hh // 2, (hh % 2) * 64
                    nc.vector.tensor_mul(OTs[m][base:base + 64, :], posb, rb)

                def attn_av_fast(hh, ex):
                    # short-chain variant for the last heads: PE broadcast of
                    # 1/denom (no DMA latency) gating the oproj finishes
                    ex0, ex1, ex23 = ex
                    rhs = [ex0, ex1, ex23[:, 0:256], ex23[:, 256:384]]
                    po = ps_ot.tile([65, T], F32, tag="po")
                    for j in range(4):
                        nc.tensor.matmul(po[:, j * 128:T],
                                         vext[j][:, 65 * hh:65 * hh + 65],
                                         rhs[j],
                                         start=(j == 0), stop=(j == 3))
                    rs = sm.tile([1, T], BF16, tag="rs")
                    with nc.allow_low_precision("softmax denom recip in bf16"):
                        nc.vector.reciprocal(out=rs, in_=po[64:65, :])
                    return po, rs

                def attn_div_fast(hh, po, rs):
                    m, base = hh // 2, (hh % 2) * 64
                    pbc = ps_bc.tile([64, T], F32, tag="pbc")
                    nc.tensor.matmul(pbc, ones1[0:1, 0:64], rs,
                                     start=True, stop=True)
                    rbt = sm.tile([64, T], BF16, tag="rbt")
                    nc.vector.tensor_copy(rbt, pbc)
                    nc.vector.tensor_mul(OTs[m][base:base + 64, :], po[0:64, :], rbt)

                def oproj_start(i, pool):
                    # pp2/pp3 borrow the (idle by then) score-tile ring so all
                    # four k=0..2 accumulations can run during the attn tail.
                    pp = pool.tile([128, D], F32, tag="pmm" if pool is ps_mm else "st")
                    for k in range(3):
                        nc.tensor.matmul(pp, OTs[k][:, i * 128:(i + 1) * 128],
                                         wo[:, k * 512:(k + 1) * 512],
                                         start=(k == 0), stop=False)
                    return pp

                def oproj_finish(i, pp):
                    nc.tensor.matmul(pp, OTs[3][:, i * 128:(i + 1) * 128],
                                     wo[:, 1536:2048], start=False,
                                     stop=not has_bias)
                    if has_bias:
                        nc.tensor.matmul(pp, ones1,
                                         rowb[:, 1536 * l + 512:1536 * l + 1024],
                                         start=False, stop=True)
                    ln_chunk(i, "w", wT, pp=pp)

                pe_touch(wqk)
                # Hybrid softmax-divide pipeline:
                #  heads 0-3 (OTs[0],[1]): DMA-broadcast path, div lag 5
                #  heads 4-7 (OTs[2],[3]): short PE-bcast path, div lag 3
                # oproj starts (k=0..2) strictly after div(0..3)+div_fast(4,5).
                scored = {}
                aved = {}
                for hd in range(8):
                    if hd % 2 == 0:
                        qk_chain(hd // 2)
                        qk_chain(hd // 2 + 4)
                    scored[hd] = attn_scores(hd)
                    if hd - 2 in scored:
                        prev = hd - 2
                        if prev < 4:
                            aved[prev] = attn_av(prev, scored.pop(prev))
                        else:
                            aved[prev] = attn_av_fast(prev, scored.pop(prev))
                    if hd - 5 in aved and hd - 5 < 4:
                        attn_div(hd - 5, *aved.pop(hd - 5))
                    if hd - 3 in aved and hd - 3 >= 4:
                        attn_div_fast(hd - 3, *aved.pop(hd - 3))
                # tail
                wT = [act.tile([128, T], BF16, tag=f"wT{k}", name=f"wT{k}") for k in range(4)]
                pe_touch(wo)
                aved[6] = attn_av_fast(6, scored.pop(6))
                attn_div(3, *aved.pop(3))
                attn_div_fast(5, *aved.pop(5))
                aved[7] = attn_av_fast(7, scored.pop(7))
                pps = [oproj_start(0, ps_mm), oproj_start(1, ps_mm)]
                attn_div_fast(6, *aved.pop(6))
                pps += [oproj_start(2, ps_st), oproj_start(3, ps_st)]
                attn_div_fast(7, *aved.pop(7))

                # ---- out proj finish + residual + LN2 (per-chunk fused) ----
                for i in range(4):
                    oproj_finish(i, pps[i])

                # ---- FFN ----
                pe_touch(wf1)
                gs = []
                for m in range(16):
                    pg = ps_rot([ps_mm, ps_ot], m)
                    for k in range(4):
                        nc.tensor.matmul(pg, wf1[:, (k * 16 + m) * 128:(k * 16 + m + 1) * 128],
                                         wT[k], start=(k == 0), stop=(k == 3))
                    g = act.tile([128, T], BF16, tag=f"g{m}")
                    nc.scalar.activation(out=g, in_=pg, func=GELU,
                                         bias=f1b[:, 16 * l + m:16 * l + m + 1], scale=1.0)
                    gs.append(g)
                # ---- ff2 + residual + next-layer LN1 (per-chunk fused) ----
                pe_touch(wf2)
                uT = [act.tile([128, T], BF16, tag=f"uT{k}", name=f"uT{k}") for k in range(4)]
                for i in range(4):
                    pf = ps_rot([ps_mm, ps_ot], i)
                    for k in range(16):
                        nc.tensor.matmul(pf, gs[k][:, i * 128:(i + 1) * 128],
                                         wf2[:, k * 512:(k + 1) * 512],
                                         start=(k == 0),
                                         stop=(not has_bias and k == 15))
                    if has_bias:
                        nc.tensor.matmul(pf, ones1,
                                         rowb[:, 1536 * l + 1024:1536 * l + 1536],
                                         start=False, stop=True)
                    ln_chunk(i, "u", uT, pp=pf)

            # ---- vocab head (uT == final-LN hidden, transposed) ----
            # Copies + out-DMAs alternate DVE/ACT; each queue issues the DMA
            # for its own copy so no queue blocks on another engine's work.
            hfT = uT
            for c in range(NC_CHUNKS):
                hw = hwp.tile([128, 2048], BF16, tag="hw")
                nc.sync.dma_start(out=hw, in_=hw_in[c])
                for i in range(4):
                    pl = ps_rot([ps_mm, ps_st, ps_ot], 4 * c + i)
                    for k in range(4):
                        nc.tensor.matmul(pl, hfT[k][:, i * 128:(i + 1) * 128],
                                         hw[:, k * 512:(k + 1) * 512],
                                         start=(k == 0), stop=(k == 3))
                    so = stg.tile([128, 512], BF16, tag="so")
                    dst = out[i * 128:(i + 1) * 128, c * 512:(c + 1) * 512]
                    if i % 2 == 0:
                        nc.vector.tensor_copy(so, pl)
                        nc.gpsimd.dma_start(out=dst, in_=so)
                    else:
                        nc.scalar.copy(out=so, in_=pl)
                        nc.scalar.dma_start(out=dst, in_=so)
    n = _split_waits(nc)
    print("split", n, "waits")
    return nc


def _split_waits(nc):
    from concourse import mybir
    SAFE = {"InstDMACopy", "InstDmaTrigger", "InstCompareAndBranch",
            "InstAllEngineBarrier", "InstNoOp", "InstEventSemaphore", "InstHalt",
            "InstBranchHint", "InstDMA", "InstISA",
            "InstCustomDveAnt"}
    DMAS = {"InstDMACopy", "InstDMA", "InstDmaTransposeAnt"}
    cnt = 0
    for f in nc.m.functions:
        for b in f.blocks:
            new = []
            for inst in b.instructions:
                tn = type(inst).__name__
                si = inst.sync_info
                w = list(si.on_wait) if si is not None and si.on_wait else []
                cap = 1
                if (len(w) > cap and tn not in (SAFE - DMAS)
                        and "bass_isa" not in type(inst).__module__):
                    for extra in w[:-cap]:
                        cnt += 1
                        new.append(mybir.InstEventSemaphore(
                            name=f"I-wsplit-{cnt}",
                            engine=inst.engine,
                            sync_info=mybir.SyncInfo(on_wait=[extra], on_update=[]),
                        ))
                    inst.sync_info = mybir.SyncInfo(
                        on_wait=w[-cap:], on_update=list(si.on_update or []))
                new.append(inst)
            b.instructions = new
    return cnt


def _prep(inputs):
    f32 = np.float32
    x = np.asarray(inputs["x"])
    tok = np.asarray(inputs["tok_emb"], f32)
    pos = np.asarray(inputs["pos_emb"], f32)
    qkv_w = np.asarray(inputs["qkv_w"], f32); qkv_b = np.asarray(inputs["qkv_b"], f32)
    out_w = np.asarray(inputs["out_w"], f32); out_b = np.asarray(inputs["out_b"], f32)
    ln1_s = np.asarray(inputs["ln1_s"], f32); ln1_b = np.asarray(inputs["ln1_b"], f32)
    ff1_w = np.asarray(inputs["ff1_w"], f32); ff1_b = np.asarray(inputs["ff1_b"], f32)
    ff2_w = np.asarray(inputs["ff2_w"], f32); ff2_b = np.asarray(inputs["ff2_b"], f32)
    ln2_s = np.asarray(inputs["ln2_s"], f32); ln2_b = np.asarray(inputs["ln2_b"], f32)
    lnf_s = np.asarray(inputs["lnf_s"], f32); lnf_b = np.asarray(inputs["lnf_b"], f32)
    head_w = np.asarray(inputs["head_w"], f32); head_b = np.asarray(inputs["head_b"], f32)

    wq = qkv_w * ln1_s[:, :, None]              # LN scale folded
    bq = qkv_b + np.einsum("ld,ldk->lk", ln1_b, qkv_w)
    wf = ff1_w * ln2_s[:, :, None]
    bf = ff1_b + np.einsum("ld,ldk->lk", ln2_b, ff1_w)
    hwf = head_w * lnf_s[:, None]
    hb_host = head_b + lnf_b @ head_w

    wqk = np.ascontiguousarray(
        wq[:, :, :1024].reshape(L, 4, 128, 8, 128).transpose(0, 2, 1, 3, 4)
    ).reshape(L, 128, 4096).astype(nbf)
    wvv = np.ascontiguousarray(
        wq[:, :, 1024:1536].reshape(L, 4, 128, 512).transpose(0, 2, 1, 3)
    ).reshape(L, 128, 2048).astype(nbf)
    woo = np.ascontiguousarray(
        out_w.reshape(L, 4, 128, 512).transpose(0, 2, 1, 3)
    ).reshape(L, 128, 2048).astype(nbf)
    wf1 = np.ascontiguousarray(
        wf.reshape(L, 4, 128, 16, 128).transpose(0, 2, 1, 3, 4)
    ).reshape(L, 128, 8192).astype(nbf)
    wf2 = np.ascontiguousarray(
        ff2_w.reshape(L, 16, 128, 512).transpose(0, 2, 1, 3)
    ).reshape(L, 128, 8192).astype(nbf)

    qkb = np.ascontiguousarray(
        bq[:, :1024].reshape(L, 8, 128).transpose(2, 0, 1)).reshape(128, 8 * L)
    qkb = np.ascontiguousarray(
        bq[:, :1024].reshape(L, 8, 128).transpose(2, 0, 1)
    )  # [128, L, 8]
    qkb = qkb.reshape(128, L * 8).astype(f32)
    f1b = np.ascontiguousarray(
        bf.reshape(L, 16, 128).transpose(2, 0, 1)).reshape(128, 16 * L).astype(f32)
    rowb = np.concatenate(
        [np.concatenate([bq[l, 1024:1536], out_b[l], ff2_b[l]]) for l in range(L)]
    ).reshape(1, 1536 * L).astype(nbf)

    kk, tt = np.arange(128)[:, None], np.arange(128)[None, :]
    maskt = np.where(kk <= tt, 0.0, NEG).astype(nbf)

    hw_pad = np.zeros((D, 2 * VH), f32)
    hw_pad[:, :V] = hwf
    halves = []
    for vh in range(2):
        sl = np.zeros((D, NC_CHUNKS * 512), f32)
        sl[:, :VH] = hw_pad[:, vh * VH:(vh + 1) * VH]
        halves.append(np.ascontiguousarray(
            sl.reshape(4, 128, NC_CHUNKS, 512).transpose(2, 1, 0, 3)
        ).reshape(NC_CHUNKS, 128, 2048).astype(nbf))

    h0 = tok[x] + pos[None, :T]                  # [B, T, D] f32
    # initial LN + transpose on host (free: h0 is host-built anyway)
    m0 = h0.mean(-1, keepdims=True)
    v0 = h0.var(-1, keepdims=True)
    u0 = (h0 - m0) / np.sqrt(v0 + EPS)           # [B, T, D]
    u0T = np.ascontiguousarray(
        u0.transpose(0, 2, 1).reshape(B, 4, 128, T).transpose(0, 2, 1, 3)
    ).reshape(B, 128, 4 * T).astype(nbf)

    has_bias = bool(np.any(bq[:, 1024:1536]) or np.any(out_b) or np.any(ff2_b))
    common = dict(wqk=wqk, wv=wvv, wo=woo, wf1=wf1, wf2=wf2,
                  qkb=qkb, f1b=f1b, maskt=maskt)
    if has_bias:
        common["rowb"] = rowb
    in_maps = []
    for core in range(8):
        b, vh = core % 4, core // 4
        m = dict(common)
        m["h0"] = np.ascontiguousarray(h0[b]).astype(f32)
        m["u0T"] = u0T[b]
        m["hw"] = halves[vh]
        in_maps.append(m)
    return in_maps, hb_host, has_bias


def kernel(**inputs):
    global LAST
    from concourse.bass_utils import run_bass_kernel_spmd
    in_maps, hb_host, has_bias = _prep(inputs)
    if has_bias not in _NC:
        _NC[has_bias] = _build_nc(has_bias)
    res = run_bass_kernel_spmd(_NC[has_bias], in_maps, list(range(8)))
    LAST = res
    full = np.empty((B, T, V), np.float32)
    for b in range(B):
        full[b, :, :VH] = res.results[b]["logits"][:, :VH].astype(np.float32)
        full[b, :, VH:] = res.results[b + 4]["logits"][:, :V - VH].astype(np.float32)
    if np.any(hb_host != 0):
        full += hb_host[None, None, :]
    return full



# revision 94
# speedup vs baseline: 1.0004x; 1.0004x over previous
import sys
sys.path.insert(0, "/opt/trn_rl_repo")
import numpy as np
import ml_dtypes

V, T, D, H, L = 50257, 512, 512, 8, 6
B = 4
HD = 64
VH = 25129          # ceil(V/2); half 1 is zero-padded to VH
NC_CHUNKS = 50      # 50*512 = 25600 >= VH
EPS = 1e-5
NEG = -1.0e9

nbf = ml_dtypes.bfloat16
_NC = {}
LAST = None


def _build_nc(has_bias=True):
    import concourse.bass as bass
    import concourse.tile as tile
    from concourse import mybir
    from concourse.masks import make_identity

    F32 = mybir.dt.float32
    BF16 = mybir.dt.bfloat16
    nc = bass.Bass()

    h0_in = nc.declare_dram_parameter("h0", [T, D], F32, isOutput=False)
    u0T_in = nc.declare_dram_parameter("u0T", [128, 4 * T], BF16, isOutput=False)
    wqk_in = nc.declare_dram_parameter("wqk", [L, 128, 4096], BF16, isOutput=False)
    wv_in = nc.declare_dram_parameter("wv", [L, 128, 2048], BF16, isOutput=False)
    wo_in = nc.declare_dram_parameter("wo", [L, 128, 2048], BF16, isOutput=False)
    wf1_in = nc.declare_dram_parameter("wf1", [L, 128, 8192], BF16, isOutput=False)
    wf2_in = nc.declare_dram_parameter("wf2", [L, 128, 8192], BF16, isOutput=False)
    qkb_in = nc.declare_dram_parameter("qkb", [128, 8 * L], F32, isOutput=False)
    f1b_in = nc.declare_dram_parameter("f1b", [128, 16 * L], F32, isOutput=False)
    rowb_in = (nc.declare_dram_parameter("rowb", [1, 1536 * L], BF16, isOutput=False)
               if has_bias else None)
    maskt_in = nc.declare_dram_parameter("maskt", [128, 128], BF16, isOutput=False)
    hw_in = nc.declare_dram_parameter("hw", [NC_CHUNKS, 128, 2048], BF16, isOutput=False)
    out = nc.declare_dram_parameter("logits", [T, NC_CHUNKS * 512], BF16, isOutput=True)

    EXP = mybir.ActivationFunctionType.Exp
    GELU = mybir.ActivationFunctionType.Gelu
    IDN = mybir.ActivationFunctionType.Identity
    SQRT = mybir.ActivationFunctionType.Sqrt

    with tile.TileContext(nc) as tc:
        with (
            tc.tile_pool(name="pers", bufs=1) as pers,
            tc.tile_pool(name="wpool", bufs=2) as wpool,
            tc.tile_pool(name="wpool1", bufs=1) as wpool1,
            tc.tile_pool(name="act", bufs=1) as act,
            tc.tile_pool(name="sm", bufs=3) as sm,
            tc.tile_pool(name="exp_pool", bufs=4) as exp_pool,
            tc.tile_pool(name="hwp", bufs=4) as hwp,
            tc.tile_pool(name="stg", bufs=8) as stg,
            tc.tile_pool(name="drp", bufs=4, space="DRAM") as drp,
            tc.tile_pool(name="ps_mm", bufs=2, space="PSUM") as ps_mm,
            tc.tile_pool(name="ps_st", bufs=2, space="PSUM") as ps_st,
            tc.tile_pool(name="ps_ot", bufs=2, space="PSUM") as ps_ot,
            tc.tile_pool(name="ps_tr", bufs=1, space="PSUM") as ps_tr,
            tc.tile_pool(name="ps_bc", bufs=1, space="PSUM") as ps_bc,
        ):
            # ---- persistent constants ----
            ident = pers.tile([128, 128], BF16, tag="ident")
            make_identity(nc, ident)
            ones1 = pers.tile([1, 128], BF16, tag="ones1")
            nc.vector.memset(ones1, 1.0)
            epst = pers.tile([128, 1], F32, tag="epst")
            nc.vector.memset(epst, EPS)
            u0big = act.tile([128, 4 * T], BF16, tag="u0big")
            nc.sync.dma_start(out=u0big, in_=u0T_in[:])
            maskt = pers.tile([128, 128], BF16, tag="maskt")
            nc.sync.dma_start(out=maskt, in_=maskt_in[:])
            qkb = pers.tile([128, 8 * L], F32, tag="qkb")
            nc.sync.dma_start(out=qkb, in_=qkb_in[:])
            f1b = pers.tile([128, 16 * L], F32, tag="f1b")
            nc.sync.dma_start(out=f1b, in_=f1b_in[:])
            if has_bias:
                rowb = pers.tile([1, 1536 * L], BF16, tag="rowb")
                nc.sync.dma_start(out=rowb, in_=rowb_in[:])
            junk_sb = pers.tile([1, 8], F32, tag="junk_sb")

            # residual + v_ext persistent (h0 DMAs deferred into layer 0 --
            # first h use is the layer-0 out-proj residual)
            h = [pers.tile([128, D], F32, tag=f"h{i}", name=f"h{i}") for i in range(4)]
            vext = [pers.tile([128, 520], BF16, tag=f"vext{i}", name=f"vext{i}") for i in range(4)]
            for i in range(4):
                nc.vector.memset(
                    vext[i].rearrange("p (h c) -> p h c", h=H)[:, :, 64:65], 1.0)

            # pre-touches: ACT reads the bias tiles once; PE touches dma'd mats
            nc.scalar.copy(out=junk_sb[0:1, 0:1], in_=qkb[0:1, 0:1])
            nc.scalar.copy(out=junk_sb[0:1, 1:2], in_=f1b[0:1, 1:2])

            def pe_touch(ap):
                jk = ps_mm.tile([1, 8], F32, tag="pmm")
                nc.tensor.matmul(jk[0:1, 0:1], ap[:, 0:1], ap[:, 0:1],
                                 start=True, stop=True, skip_group_check=True)

            if has_bias:
                pe_touch(rowb)

            PTAG = {id(ps_mm): "pmm", id(ps_st): "st", id(ps_ot): "po", id(ps_tr): "tp"}

            def ps_rot(pools, n):
                # rotate full-bank [128,D] f32 psum tiles across idle pools
                pool = pools[n % len(pools)]
                return pool.tile([128, D], F32, tag=PTAG[id(pool)], name="prot")

            LN_ = mybir.ActivationFunctionType.Ln
            # NOTE: GPSIMD/Pool cannot access PSUM on HW — PSUM-sourced
            # copies must go to DVE or ACT.
            cp_engines = [
                lambda o, i_: nc.vector.tensor_copy(o, i_),
                lambda o, i_: nc.scalar.copy(out=o, in_=i_),
            ]

            def ln_chunk(i, utag, uT, pp=None):
                # h[i] (+ optional residual pp) -> normalized, transposed
                # into uT. rstd = exp(-0.5*ln(var+eps)) keeps ACT on one
                # function table; the normalize is split DVE/ACT so the
                # first transposes start earlier.
                if pp is not None:
                    nc.vector.tensor_add(h[i], h[i], pp)
                src = h[i]
                stats = sm.tile([128, 6], F32, tag="stats")
                mv = sm.tile([128, 2], F32, tag="mv")
                nc.vector.bn_stats(out=stats, in_=src)
                nc.vector.bn_aggr(out=mv, in_=stats)
                vin = mv[:, 1:2]
                lnv = sm.tile([128, 1], F32, tag="lnv")
                nc.scalar.activation(out=lnv, in_=vin, func=LN_,
                                     bias=epst, scale=1.0)
                rstd = sm.tile([128, 1], F32, tag="rstd")
                nc.scalar.activation(out=rstd, in_=lnv, func=EXP, scale=-0.5)
                u = act.tile([128, D], BF16, tag=f"{utag}{i}")
                nc.vector.tensor_scalar(
                    out=u[:, 0:256], in0=src[:, 0:256], scalar1=mv[:, 0:1],
                    scalar2=rstd,
                    op0=mybir.AluOpType.subtract, op1=mybir.AluOpType.mult)
                nmr = sm.tile([128, 1], F32, tag="nmr")
                nc.vector.tensor_scalar(
                    out=nmr, in0=mv[:, 0:1], scalar1=rstd, scalar2=-1.0,
                    op0=mybir.AluOpType.mult, op1=mybir.AluOpType.mult)
                nc.scalar.activation(out=u[:, 256:512], in_=src[:, 256:512],
                                     func=IDN, bias=nmr, scale=rstd)
                for k in range(4):
                    # alternate PSUM banks (ps_bc is idle outside attention)
                    # so transpose k+1 does not wait on copy k
                    pool, tag = (ps_tr, "tp") if k % 2 == 0 else (ps_bc, "pbc")
                    tp = pool.tile([128, 128], BF16, tag=tag)
                    nc.tensor.transpose(tp, u[:, k * 128:(k + 1) * 128], ident)
                    cp_engines[k % 2](uT[k][:, i * 128:(i + 1) * 128], tp)

            # layer-0 uT comes pre-normalized+transposed from the host
            uT = [u0big[:, k * T:(k + 1) * T] for k in range(4)]

            for l in range(L):
                # ---- stream layer weights (SP queue; prefetch-friendly) ----
                wv = wpool1.tile([128, 2048], BF16, tag="wv")
                nc.sync.dma_start(out=wv, in_=wv_in[l])
                wqk = wpool.tile([128, 4096], BF16, tag="wqk")
                nc.sync.dma_start(out=wqk, in_=wqk_in[l])
                wo = wpool1.tile([128, 2048], BF16, tag="wo")
                nc.sync.dma_start(out=wo, in_=wo_in[l])
                wf1 = wpool1.tile([128, 8192], BF16, tag="wf1")
                nc.sync.dma_start(out=wf1, in_=wf1_in[l])
                wf2 = wpool1.tile([128, 8192], BF16, tag="wf2")
                nc.sync.dma_start(out=wf2, in_=wf2_in[l])
                if l == 0:
                    for i in range(4):
                        nc.sync.dma_start(out=h[i], in_=h0_in[i * 128:(i + 1) * 128, :])

                # ---- v (natural) -> vext ----
                # layer 0: the initial LN chunks interleave with the v chains
                # (v chain i needs only chunk i's transposed columns)
                pe_touch(wv)
                for i in range(4):
                    pv = ps_mm.tile([128, T], F32, tag="pmm")
                    for k in range(4):
                        nc.tensor.matmul(pv, uT[k][:, i * 128:(i + 1) * 128],
                                         wv[:, k * 512:(k + 1) * 512],
                                         start=(k == 0),
                                         stop=(not has_bias and k == 3))
                    if has_bias:
                        nc.tensor.matmul(pv, ones1, rowb[:, 1536 * l:1536 * l + 512],
                                         start=False, stop=True)
                    nc.vector.tensor_copy(
                        vext[i].rearrange("p (h c) -> p h c", h=H)[:, :, 0:64],
                        pv[:].rearrange("p (h c) -> p h c", h=H))

                # ---- qk chains interleaved with attention heads ----
                OTs = [act.tile([128, T], BF16, tag=f"ots{k}", name=f"ots{k}l") for k in range(4)]
                qTs, kTs = [None] * 4, [None] * 4

                qkn = [0]

                def qk_chain(m):
                    pq = ps_rot([ps_mm, ps_tr], qkn[0]); qkn[0] += 1
                    for k in range(4):
                        nc.tensor.matmul(pq, wqk[:, (k * 8 + m) * 128:(k * 8 + m + 1) * 128],
                                         uT[k], start=(k == 0), stop=(k == 3))
                    dst = act.tile([128, T], BF16, tag=f"qk{m}")
                    if m % 4 < 2:
                        # ACT is idle early in attention; DVE is chain-bound
                        nc.scalar.activation(out=dst, in_=pq, func=IDN,
                                             bias=qkb[:, 8 * l + m:8 * l + m + 1],
                                             scale=1.0)
                    else:
                        nc.vector.tensor_scalar_add(out=dst, in0=pq,
                                                    scalar1=qkb[:, 8 * l + m:8 * l + m + 1])
                    if m < 4:
                        qTs[m] = dst
                    else:
                        kTs[m - 4] = dst

                def attn_scores(hh):
                    # score blocks j=1 (384 cols) and j=3 (128 cols) share one
                    # PSUM tile / one exp call: 3 exps per head instead of 4.
                    m, base = hh // 2, (hh % 2) * 64
                    kT, qT = kTs[m], qTs[m]

                    def sc(st, col0, j):
                        nj = T - j * 128
                        nc.tensor.matmul(st[:, col0:col0 + nj],
                                         kT[base:base + 64, j * 128:(j + 1) * 128],
                                         qT[base:base + 64, j * 128:T],
                                         start=True, stop=False)
                        nc.tensor.matmul(st[:, col0:col0 + 128], ident, maskt,
                                         start=False, stop=True)

                    st0 = ps_st.tile([128, T], F32, tag="st")
                    sc(st0, 0, 0)
                    st1 = ps_st.tile([128, T], F32, tag="st")
                    sc(st1, 0, 1)
                    st23 = ps_st.tile([128, T], F32, tag="st")
                    sc(st23, 0, 2)
                    sc(st23, 256, 3)
                    ex0 = exp_pool.tile([128, T], BF16, tag="ex0", name="ex0")
                    nc.scalar.activation(out=ex0, in_=st0, func=EXP, scale=0.125)
                    ex1 = exp_pool.tile([128, 384], BF16, tag="ex1", name="ex1")
                    nc.scalar.activation(out=ex1, in_=st1[:, 0:384], func=EXP, scale=0.125)
                    ex23 = exp_pool.tile([128, 384], BF16, tag="ex23", name="ex23")
                    nc.scalar.activation(out=ex23, in_=st23[:, 0:384], func=EXP, scale=0.125)
                    return ex0, ex1, ex23

                def attn_av(hh, ex):
                    # av matmuls; stage O to SBUF early (frees the po bank),
                    # broadcast 1/denom via a DRAM round-trip DMA (off-engine)
                    ex0, ex1, ex23 = ex
                    rhs = [ex0, ex1, ex23[:, 0:256], ex23[:, 256:384]]
                    po = ps_ot.tile([65, T], F32, tag="po")
                    for j in range(4):
                        nc.tensor.matmul(po[:, j * 128:T],
                                         vext[j][:, 65 * hh:65 * hh + 65],
                                         rhs[j],
                                         start=(j == 0), stop=(j == 3))
                    rs = sm.tile([1, T], BF16, tag="rs")
                    with nc.allow_low_precision("softmax denom recip in bf16"):
                        nc.vector.reciprocal(out=rs, in_=po[64:65, :])
                    posb = exp_pool.tile([64, T], BF16, tag="posb")
                    nc.vector.tensor_copy(posb, po[0:64, :])
                    dsc = drp.tile([1, T], BF16, tag="dsc")
                    nc.sync.dma_start(out=dsc, in_=rs)
                    rb = exp_pool.tile([64, T], BF16, tag="rb")
                    nc.sync.dma_start(out=rb, in_=dsc.to_broadcast([64, T]))
                    return posb, rb

                def attn_div(hh, posb, rb):
                    # all-bf16 SBUF multiply (4x DVE mode)
                    m, base = hh // 2, (hh % 2) * 64
                    nc.vector.tensor_mul(OTs[m][base:base + 64, :], posb, rb)

                def attn_av_fast(hh, ex):
                    # short-chain variant for the last heads: PE broadcast of
                    # 1/denom (no DMA latency) gating the oproj finishes
                    ex0, ex1, ex23 = ex
                    rhs = [ex0, ex1, ex23[:, 0:256], ex23[:, 256:384]]
                    po = ps_ot.tile([65, T], F32, tag="po")
                    for j in range(4):
                        nc.tensor.matmul(po[:, j * 128:T],
                                         vext[j][:, 65 * hh:65 * hh + 65],
                                         rhs[j],
                                         start=(j == 0), stop=(j == 3))
                    rs = sm.tile([1, T], BF16, tag="rs")
                    with nc.allow_low_precision("softmax denom recip in bf16"):
                        nc.vector.reciprocal(out=rs, in_=po[64:65, :])
                    return po, rs

                def attn_div_fast(hh, po, rs):
                    m, base = hh // 2, (hh % 2) * 64
                    pbc = ps_bc.tile([64, T], F32, tag="pbc")
                    nc.tensor.matmul(pbc, ones1[0:1, 0:64], rs,
                                     start=True, stop=True)
                    rbt = sm.tile([64, T], BF16, tag="rbt")
                    nc.vector.tensor_copy(rbt, pbc)
                    nc.vector.tensor_mul(OTs[m][base:base + 64, :], po[0:64, :], rbt)

                def oproj_start(i, pool):
                    # pp2/pp3 borrow the (idle by then) score-tile ring so all
                    # four k=0..2 accumulations can run during the attn tail.
                    pp = pool.tile([128, D], F32, tag="pmm" if pool is ps_mm else "st")
                    for k in range(3):
                        nc.tensor.matmul(pp, OTs[k][:, i * 128:(i + 1) * 128],
                                         wo[:, k * 512:(k + 1) * 512],
                                         start=(k == 0), stop=False)
                    return pp

                def oproj_finish(i, pp):
                    nc.tensor.matmul(pp, OTs[3][:, i * 128:(i + 1) * 128],
                                     wo[:, 1536:2048], start=False,
                                     stop=not has_bias)
                    if has_bias:
                        nc.tensor.matmul(pp, ones1,
                                         rowb[:, 1536 * l + 512:1536 * l + 1024],
                                         start=False, stop=True)
                    ln_chunk(i, "w", wT, pp=pp)

                pe_touch(wqk)
                # Hybrid softmax-divide pipeline:
                #  heads 0-3 (OTs[0],[1]): DMA-broadcast path, div lag 5
                #  heads 4-7 (OTs[2],[3]): short PE-bcast path, div lag 3
                # oproj starts (k=0..2) strictly after div(0..3)+div_fast(4,5).
                scored = {}
                aved = {}
                for hd in range(8):
                    if hd % 2 == 0:
                        qk_chain(hd // 2)
                        qk_chain(hd // 2 + 4)
                    scored[hd] = attn_scores(hd)
                    if hd - 2 in scored:
                        prev = hd - 2
                        if prev < 4:
                            aved[prev] = attn_av(prev, scored.pop(prev))
                        else:
                            aved[prev] = attn_av_fast(prev, scored.pop(prev))
                    if hd - 5 in aved and hd - 5 < 4:
                        attn_div(hd - 5, *aved.pop(hd - 5))
                    if hd - 3 in aved and hd - 3 >= 4:
                        attn_div_fast(hd - 3, *aved.pop(hd - 3))
                # tail
                wT = [act.tile([128, T], BF16, tag=f"wT{k}", name=f"wT{k}") for k in range(4)]
                pe_touch(wo)
                aved[6] = attn_av_fast(6, scored.pop(6))
                attn_div(3, *aved.pop(3))
                attn_div_fast(5, *aved.pop(5))
                aved[7] = attn_av_fast(7, scored.pop(7))
                pps = [oproj_start(0, ps_mm), oproj_start(1, ps_mm)]
                attn_div_fast(6, *aved.pop(6))
                pps += [oproj_start(2, ps_st), oproj_start(3, ps_st)]
                attn_div_fast(7, *aved.pop(7))

                # ---- out proj finish + residual + LN2 (per-chunk fused) ----
                for i in range(4):
                    oproj_finish(i, pps[i])

                # ---- FFN ----
                pe_touch(wf1)
                gs = []
                for m in range(16):
                    pg = ps_rot([ps_ot, ps_mm], m)
                    for k in range(4):
                        nc.tensor.matmul(pg, wf1[:, (k * 16 + m) * 128:(k * 16 + m + 1) * 128],
                                         wT[k], start=(k == 0), stop=(k == 3))
                    g = act.tile([128, T], BF16, tag=f"g{m}")
                    nc.scalar.activation(out=g, in_=pg, func=GELU,
                                         bias=f1b[:, 16 * l + m:16 * l + m + 1], scale=1.0)
                    gs.append(g)
                # ---- ff2 + residual + next-layer LN1 (per-chunk fused) ----
                pe_touch(wf2)
                uT = [act.tile([128, T], BF16, tag=f"uT{k}", name=f"uT{k}") for k in range(4)]
                for i in range(4):
                    pf = ps_rot([ps_ot, ps_mm], i)
                    for k in range(16):
                        nc.tensor.matmul(pf, gs[k][:, i * 128:(i + 1) * 128],
                                         wf2[:, k * 512:(k + 1) * 512],
                                         start=(k == 0),
                                         stop=(not has_bias and k == 15))
                    if has_bias:
                        nc.tensor.matmul(pf, ones1,
                                         rowb[:, 1536 * l + 1024:1536 * l + 1536],
                                         start=False, stop=True)
                    ln_chunk(i, "u", uT, pp=pf)

            # ---- vocab head (uT == final-LN hidden, transposed) ----
            # Copies + out-DMAs alternate DVE/ACT; each queue issues the DMA
            # for its own copy so no queue blocks on another engine's work.
            hfT = uT
            for c in range(NC_CHUNKS):
                hw = hwp.tile([128, 2048], BF16, tag="hw")
                nc.sync.dma_start(out=hw, in_=hw_in[c])
                for i in range(4):
                    pl = ps_rot([ps_mm, ps_st, ps_ot], 4 * c + i)
                    for k in range(4):
                        nc.tensor.matmul(pl, hfT[k][:, i * 128:(i + 1) * 128],
                                         hw[:, k * 512:(k + 1) * 512],
                                         start=(k == 0), stop=(k == 3))
                    so = stg.tile([128, 512], BF16, tag="so")
                    dst = out[i * 128:(i + 1) * 128, c * 512:(c + 1) * 512]
                    if i % 2 == 0:
                        nc.vector.tensor_copy(so, pl)
                        nc.gpsimd.dma_start(out=dst, in_=so)
                    else:
                        nc.scalar.copy(out=so, in_=pl)
                        nc.scalar.dma_start(out=dst, in_=so)
    n = _split_waits(nc)
    print("split", n, "waits")
    return nc


def _split_waits(nc):
    from concourse import mybir
    SAFE = {"InstDMACopy", "InstDmaTrigger", "InstCompareAndBranch",
            "InstAllEngineBarrier", "InstNoOp", "InstEventSemaphore", "InstHalt",
            "InstBranchHint", "InstDMA", "InstISA",
            "InstCustomDveAnt"}
    DMAS = {"InstDMACopy", "InstDMA", "InstDmaTransposeAnt"}
    cnt = 0
    for f in nc.m.functions:
        for b in f.blocks:
            new = []
            for inst in b.instructions:
                tn = type(inst).__name__
                si = inst.sync_info
                w = list(si.on_wait) if si is not None and si.on_wait else []
                cap = 1
                if (len(w) > cap and tn not in (SAFE - DMAS)
                        and "bass_isa" not in type(inst).__module__):
                    for extra in w[:-cap]:
                        cnt += 1
                        new.append(mybir.InstEventSemaphore(
                            name=f"I-wsplit-{cnt}",
                            engine=inst.engine,
                            sync_info=mybir.SyncInfo(on_wait=[extra], on_update=[]),
                        ))
                    inst.sync_info = mybir.SyncInfo(
                        on_wait=w[-cap:], on_update=list(si.on_update or []))
                new.append(inst)
            b.instructions = new
    return cnt


def _prep(inputs):
    f32 = np.float32
    x = np.asarray(inputs["x"])
    tok = np.asarray(inputs["tok_emb"], f32)
    pos = np.asarray(inputs["pos_emb"], f32)
    qkv_w = np.asarray(inputs["qkv_w"], f32); qkv_b = np.asarray(inputs["qkv_b"], f32)
    out_w = np.asarray(inputs["out_w"], f32); out_b = np.asarray(inputs["out_b"], f32)
    ln1_s = np.asarray(inputs["ln1_s"], f32); ln1_b = np.asarray(inputs["ln1_b"], f32)
    ff1_w = np.asarray(inputs["ff1_w"], f32); ff1_b = np.asarray(inputs["ff1_b"], f32)
    ff2_w = np.asarray(inputs["ff2_w"], f32); ff2_b = np.asarray(inputs["ff2_b"], f32)
    ln2_s = np.asarray(inputs["ln2_s"], f32); ln2_b = np.asarray(inputs["ln2_b"], f32)
    lnf_s = np.asarray(inputs["lnf_s"], f32); lnf_b = np.asarray(inputs["lnf_b"], f32)
    head_w = np.asarray(inputs["head_w"], f32); head_b = np.asarray(inputs["head_b"], f32)

    wq = qkv_w * ln1_s[:, :, None]              # LN scale folded
    bq = qkv_b + np.einsum("ld,ldk->lk", ln1_b, qkv_w)
    wf = ff1_w * ln2_s[:, :, None]
    bf = ff1_b + np.einsum("ld,ldk->lk", ln2_b, ff1_w)
    hwf = head_w * lnf_s[:, None]
    hb_host = head_b + lnf_b @ head_w

    wqk = np.ascontiguousarray(
        wq[:, :, :1024].reshape(L, 4, 128, 8, 128).transpose(0, 2, 1, 3, 4)
    ).reshape(L, 128, 4096).astype(nbf)
    wvv = np.ascontiguousarray(
        wq[:, :, 1024:1536].reshape(L, 4, 128, 512).transpose(0, 2, 1, 3)
    ).reshape(L, 128, 2048).astype(nbf)
    woo = np.ascontiguousarray(
        out_w.reshape(L, 4, 128, 512).transpose(0, 2, 1, 3)
    ).reshape(L, 128, 2048).astype(nbf)
    wf1 = np.ascontiguousarray(
        wf.reshape(L, 4, 128, 16, 128).transpose(0, 2, 1, 3, 4)
    ).reshape(L, 128, 8192).astype(nbf)
    wf2 = np.ascontiguousarray(
        ff2_w.reshape(L, 16, 128, 512).transpose(0, 2, 1, 3)
    ).reshape(L, 128, 8192).astype(nbf)

    qkb = np.ascontiguousarray(
        bq[:, :1024].reshape(L, 8, 128).transpose(2, 0, 1)).reshape(128, 8 * L)
    qkb = np.ascontiguousarray(
        bq[:, :1024].reshape(L, 8, 128).transpose(2, 0, 1)
    )  # [128, L, 8]
    qkb = qkb.reshape(128, L * 8).astype(f32)
    f1b = np.ascontiguousarray(
        bf.reshape(L, 16, 128).transpose(2, 0, 1)).reshape(128, 16 * L).astype(f32)
    rowb = np.concatenate(
        [np.concatenate([bq[l, 1024:1536], out_b[l], ff2_b[l]]) for l in range(L)]
    ).reshape(1, 1536 * L).astype(nbf)

    kk, tt = np.arange(128)[:, None], np.arange(128)[None, :]
    maskt = np.where(kk <= tt, 0.0, NEG).astype(nbf)

    hw_pad = np.zeros((D, 2 * VH), f32)
    hw_pad[:, :V] = hwf
    halves = []
    for vh in range(2):
        sl = np.zeros((D, NC_CHUNKS * 512), f32)
        sl[:, :VH] = hw_pad[:, vh * VH:(vh + 1) * VH]
        halves.append(np.ascontiguousarray(
            sl.reshape(4, 128, NC_CHUNKS, 512).transpose(2, 1, 0, 3)
        ).reshape(NC_CHUNKS, 128, 2048).astype(nbf))

    h0 = tok[x] + pos[None, :T]                  # [B, T, D] f32
    # initial LN + transpose on host (free: h0 is host-built anyway)
    m0 = h0.mean(-1, keepdims=True)
    v0 = h0.var(-1, keepdims=True)
    u0 = (h0 - m0) / np.sqrt(v0 + EPS)           # [B, T, D]
    u0T = np.ascontiguousarray(
        u0.transpose(0, 2, 1).reshape(B, 4, 128, T).transpose(0, 2, 1, 3)
    ).reshape(B, 128, 4 * T).astype(nbf)

    has_bias = bool(np.any(bq[:, 1024:1536]) or np.any(out_b) or np.any(ff2_b))
    common = dict(wqk=wqk, wv=wvv, wo=woo, wf1=wf1, wf2=wf2,
                  qkb=qkb, f1b=f1b, maskt=maskt)
    if has_bias:
        common["rowb"] = rowb
    in_maps = []
    for core in range(8):
        b, vh = core % 4, core // 4
        m = dict(common)
        m["h0"] = np.ascontiguousarray(h0[b]).astype(f32)
        m["u0T"] = u0T[b]
        m["hw"] = halves[vh]
        in_maps.append(m)
    return in_maps, hb_host, has_bias


def kernel(**inputs):
    global LAST
    from concourse.bass_utils import run_bass_kernel_spmd
    in_maps, hb_host, has_bias = _prep(inputs)
    if has_bias not in _NC:
        _NC[has_bias] = _build_nc(has_bias)
    res = run_bass_kernel_spmd(_NC[has_bias], in_maps, list(range(8)))
    LAST = res
    full = np.empty((B, T, V), np.float32)
    for b in range(B):
        full[b, :, :VH] = res.results[b]["logits"][:, :VH].astype(np.float32)
        full[b, :, VH:] = res.results[b + 4]["logits"][:, :V - VH].astype(np.float32)
    if np.any(hb_host != 0):
        full += hb_host[None, None, :]
    return full

